# revision 34
# baseline (speedup 1.0000x reference)
"""Multi-head causal self-attention with RoPE on 8 Trainium2 NeuronCores.

Sharding: DP(2) x TP(4). Cores [4g, 4g+4) own batch g; within a group,
core r owns heads [4r, 4r+4) (rows [r*512,(r+1)*512) of Wq/Wk/Wv and the
matching columns of Wo). The host sums the 4 partial output projections
per batch (replaces the TP all-reduce); partial sums travel as fp16.

Performance notes (measured on TRN2):
  - PE matmul issue rate is N cycles @2.4GHz regardless of operand dtype
    (fp16 == bf16 == weight-reuse); the kernel is PE-streaming-bound, so
    everything else is organized to keep the PE FIFO dense.
  - dma_start issue on the Sync engine costs ~2.7ns per descriptor line;
    all DRAM tensors are pre-tiled on the host so every transfer is 128
    long per-partition-contiguous descriptors (~0.35us issue each).
  - Scalar activations pay a ~370-cycle access-latency adder, so exps are
    batched two k-chunks per call (st pairs span 2 PSUM banks; engines
    other than the PE may read across banks).
  - Softmax denominators: all exp'd chunks of a (head, q-chunk) chain
    are accumulated in fp16 on the DVE (2x mode), then partition-reduced
    with a single ones-matmul; the reduce/reciprocal/normalize epilogue
    is lagged into the next chain to stay off the PE critical path.
  - Causality: diagonal-band score chunks are column-trimmed to
    N = 512-128*di and masked multiplicatively (after exp) with a single
    [128,128] triangular 0/1 mask.
"""

import sys

import numpy as np

B, S, DIM = 2, 2048, 2048
NUM_HEADS = 16
HD = 128
N_CORES = 8
DP = 2                       # data-parallel groups (one batch each)
TP = N_CORES // DP           # tensor-parallel ranks per group
HPC = NUM_HEADS // TP        # heads per core (4)
DLOC = HPC * HD              # per-core slice of the model dim (512)
ROPE_BASE = 10000.0
SC = 512                     # s-chunk for projections / attention q-chunk

_PROGRAM_CACHE = {}


def _rope_tables_T(seq_len, head_dim):
    # match reference float32 arithmetic: inv_freq over even indices,
    # emb = cat(freqs, freqs); returned transposed [head_dim, seq_len]
    inv_freq = (
        1.0
        / (np.float32(ROPE_BASE)
           ** (np.arange(0, head_dim, 2, dtype=np.float32) / np.float32(head_dim)))
    ).astype(np.float32)
    t = np.arange(seq_len, dtype=np.float32)
    freqs = np.outer(t, inv_freq).astype(np.float32)      # [S, D/2]
    emb = np.concatenate([freqs, freqs], axis=-1)         # [S, D]
    return (
        np.ascontiguousarray(np.cos(emb).astype(np.float16).T),
        np.ascontiguousarray(np.sin(emb).astype(np.float16).T),
    )


def _rot_matrix_T(head_dim):
    # rotated = cat(-x[1::2], x[::2]) = R @ x; return R.T [D, D]
    d2 = head_dim // 2
    R = np.zeros((head_dim, head_dim), dtype=np.float16)
    for dp in range(d2):
        R[dp, 2 * dp + 1] = -1.0
    for dp in range(d2, head_dim):
        R[dp, 2 * (dp - d2)] = 1.0
    return np.ascontiguousarray(R.T)


def _tri01():
    # tri01[kk, qq] = 1 if kk <= qq else 0 (multiplicative causal mask for
    # the [128,128] diagonal block of every diagonal k-chunk)
    kk = np.arange(128)[:, None]
    qq = np.arange(128)[None, :]
    return np.ascontiguousarray((kk <= qq).astype(np.float16))


def build_program(s=S, dim=DIM):
    """Per-core SPMD Bass program (identical on every core)."""
    if "/opt/trn_rl_repo" not in sys.path:
        sys.path.insert(0, "/opt/trn_rl_repo")
    import concourse.bacc as bacc
    import concourse.mybir as mybir
    import concourse.tile as tile

    f32 = mybir.dt.float32
    f16 = mybir.dt.float16
    EXP = mybir.ActivationFunctionType.Exp

    n_din = dim // 128          # contraction chunks for projections (16)
    n_sc = s // SC              # s-chunks (4)
    n_oc = dim // 128           # output-projection row chunks (16)
    scale = float(HD) ** -0.5

    nc = bacc.Bacc("TRN2", target_bir_lowering=False, debug=False)

    # all DRAM tensors pre-tiled on the host: partition dim first, then
    # per-partition-contiguous free dims, so DMAs are 128 fat descriptors
    x_d = nc.dram_tensor("x", [128, n_sc, n_din, SC], f16, kind="ExternalInput")
    wq_d = nc.dram_tensor("wq", [128, HPC, n_din, HD], f16, kind="ExternalInput")
    wk_d = nc.dram_tensor("wk", [128, HPC, n_din, HD], f16, kind="ExternalInput")
    wv_d = nc.dram_tensor("wv", [128, n_din, DLOC], f16, kind="ExternalInput")
    wo_d = nc.dram_tensor("wo", [128, HPC, dim], f16, kind="ExternalInput")
    cosT_d = nc.dram_tensor("cosT", [HD, s], f16, kind="ExternalInput")
    sinT_d = nc.dram_tensor("sinT", [HD, s], f16, kind="ExternalInput")
    rT_d = nc.dram_tensor("rT", [HD, HD], f16, kind="ExternalInput")
    ones_d = nc.dram_tensor("ones", [HD, HD], f16, kind="ExternalInput")
    tri_d = nc.dram_tensor("tri", [HD, HD], f16, kind="ExternalInput")
    out_d = nc.dram_tensor("out", [128, n_sc, n_oc, SC], f16, kind="ExternalOutput")

    with tile.TileContext(nc) as tc:
        with tc.tile_pool(name="persist", bufs=1) as persist:
            qT = persist.tile([128, HPC, s], f16)   # roped q, [d, h, s]
            kT = persist.tile([128, HPC, s], f16)
            vS = persist.tile([128, s // 128, DLOC], f16)  # [k, chunk, d]
            uT = persist.tile([128, HPC, s], f16)   # attention out, [d, h, s]

            # ---------------- phase 1: qkv projections + RoPE ----------------
            with (
                tc.tile_pool(name="p1x", bufs=3) as p1x,
                tc.tile_pool(name="p1w", bufs=1) as p1w,
                tc.tile_pool(name="p1t", bufs=2) as p1t,
                tc.tile_pool(name="ps1", bufs=2, space="PSUM") as ps1,
            ):
                # startup criticality order: chain (q,h=0) is paced by xt0's
                # c-groups and wq's h=0 slice; later pieces arrive while
                # earlier chains compute
                xts = [
                    p1x.tile([128, n_din, SC], f16, tag="xt", name=f"xt{si}")
                    for si in range(3)
                ]
                wq_s = p1w.tile([128, HPC, n_din, HD], f16)
                wk_s = p1w.tile([128, HPC, n_din, HD], f16)
                wv_s = p1w.tile([128, n_din, DLOC], f16)
                rTs = persist.tile([HD, HD], f16)
                cosT = persist.tile([HD, s], f16)
                sinT = persist.tile([HD, s], f16)
                gq = n_din // 4
                nc.sync.dma_start(out=xts[0][:, :gq, :], in_=x_d[:, 0, :gq, :])
                nc.sync.dma_start(out=wq_s[:, 0, :, :], in_=wq_d[:, 0, :, :])
                for g0 in range(gq, n_din, gq):
                    sl4 = slice(g0, g0 + gq)
                    nc.sync.dma_start(out=xts[0][:, sl4, :], in_=x_d[:, 0, sl4, :])
                for h in range(1, HPC):
                    nc.sync.dma_start(out=wq_s[:, h, :, :], in_=wq_d[:, h, :, :])
                nc.sync.dma_start(out=rTs, in_=rT_d[:])
                nc.sync.dma_start(out=cosT, in_=cosT_d[:])
                nc.sync.dma_start(out=sinT, in_=sinT_d[:])
                for h in range(HPC):
                    nc.sync.dma_start(out=wk_s[:, h, :, :], in_=wk_d[:, h, :, :])
                nh = n_din // 2
                nc.sync.dma_start(out=xts[1][:, :nh, :], in_=x_d[:, 1, :nh, :])
                nc.sync.dma_start(out=xts[1][:, nh:, :], in_=x_d[:, 1, nh:, :])
                nc.sync.dma_start(out=wv_s[:, :nh, :], in_=wv_d[:, :nh, :])
                nc.sync.dma_start(out=wv_s[:, nh:, :], in_=wv_d[:, nh:, :])
                nc.sync.dma_start(out=xts[2][:, :nh, :], in_=x_d[:, 2, :nh, :])
                nc.sync.dma_start(out=xts[2][:, nh:, :], in_=x_d[:, 2, nh:, :])
                ones = persist.tile([HD, HD], f16)
                nc.sync.dma_start(out=ones, in_=ones_d[:])
                tri01 = persist.tile([HD, HD], f16)
                nc.sync.dma_start(out=tri01, in_=tri_d[:])
                woT_s = persist.tile([128, HPC, dim], f16)
                nc.sync.dma_start(out=woT_s, in_=wo_d[:])

                def finish_rope(raw, store, h, s0):
                    # rot matmul emitted one chain late so the PE never waits
                    # on the scalar-engine raw copy
                    rot = ps1.tile([128, SC], f32, tag="rot")
                    nc.tensor.matmul(rot, lhsT=rTs, rhs=raw, start=True, stop=True)
                    t1 = p1t.tile([128, SC], f16, tag="t1")
                    nc.vector.tensor_mul(t1, raw, cosT[:, s0 : s0 + SC])
                    t2 = p1t.tile([128, SC], f16, tag="t2")
                    nc.vector.tensor_mul(t2, rot, sinT[:, s0 : s0 + SC])
                    nc.gpsimd.tensor_add(store[:, h, s0 : s0 + SC], t1, t2)

                pending = None
                for si in range(n_sc):
                    s0 = si * SC
                    if si < 3:
                        xt = xts[si]
                    else:
                        xt = p1x.tile([128, n_din, SC], f16, tag="xt", name="xt3")
                        nc.sync.dma_start(out=xt, in_=x_d[:, si, :, :])

                    for w_s, store in ((wq_s, qT), (wk_s, kT)):
                        for h in range(HPC):
                            acc = ps1.tile([128, SC], f32, tag="acc")
                            for c in range(n_din):
                                nc.tensor.matmul(
                                    acc,
                                    lhsT=w_s[:, h, c, :],
                                    rhs=xt[:, c, :],
                                    start=(c == 0),
                                    stop=(c == n_din - 1),
                                )
                            raw = p1t.tile([128, SC], f16, tag="raw", bufs=3)
                            nc.scalar.copy(raw, acc)
                            if pending is not None:
                                finish_rope(*pending)
                            pending = (raw, store, h, s0)

                    for sub in range(SC // 128):   # v chains
                        vacc = ps1.tile([128, SC], f32, tag="vacc")
                        for c in range(n_din):
                            nc.tensor.matmul(
                                vacc,
                                lhsT=xt[:, c, sub * 128 : (sub + 1) * 128],
                                rhs=wv_s[:, c, :],
                                start=(c == 0),
                                stop=(c == n_din - 1),
                            )
                        if pending is not None:
                            finish_rope(*pending)
                            pending = None
                        vdst = vS[:, si * 4 + sub, :]
                        if sub % 2 == 0:
                            nc.scalar.copy(vdst, vacc)
                        else:
                            nc.vector.tensor_copy(vdst, vacc)

            # ------------- phase 2+3: attention + output projection -------------
            with (
                tc.tile_pool(name="p2", bufs=8) as p2,
                tc.tile_pool(name="p2l", bufs=2) as p2l,
                tc.tile_pool(name="p2r", bufs=2) as p2r,
                tc.tile_pool(name="p3", bufs=2) as p3,
                tc.tile_pool(name="ps_t", bufs=3, space="PSUM") as ps_t,
                tc.tile_pool(name="ps_o", bufs=2, space="PSUM") as ps_o,
            ):
                def phase3_og_gen(qc, og):
                    # one 4-row-chunk group of the output projection for
                    # q-chunk qc: 16 PE matmuls + 2 psum-pair copies + 1 DMA.
                    # Yields after each 4-matmul row-chunk so the group can be
                    # dosed through the next q-chunk's attention chains --
                    # keeping per-pair PE work above the scalar exp rate.
                    q0 = qc * SC
                    ot4 = p3.tile([128, 4, SC], f16, tag="ot")
                    for op_ in range(2):
                        pos2 = ps_t.tile([128, 2, SC], f32, tag="st", name="pos")
                        for j in range(2):
                            oc = og * 4 + op_ * 2 + j
                            for h in range(HPC):
                                nc.tensor.matmul(
                                    pos2[:, j, :],
                                    lhsT=woT_s[:, h, oc * 128 : (oc + 1) * 128],
                                    rhs=uT[:, h, q0 : q0 + SC],
                                    start=(h == 0),
                                    stop=(h == HPC - 1),
                                )
                            # per-row-chunk psum evictions, alternating
                            # engines: half-size ops block the latency-
                            # critical exps/masks in the FIFOs half as long
                            dst = ot4[:, op_ * 2 + j, :]
                            if (op_ * 2 + j) % 2 == 0:
                                nc.scalar.copy(dst, pos2[:, j, :])
                            else:
                                nc.vector.tensor_copy(dst, pos2[:, j, :])
                            yield
                    nc.sync.dma_start(
                        out=out_d[:, qc, og * 4 : (og + 1) * 4, :], in_=ot4
                    )

                def phase3_og(qc, og):
                    for _ in phase3_og_gen(qc, og):
                        pass

                # attention q-chunks processed in rotated order so every
                # chain (including the short qc=0 ones) carries an output-
                # projection filler group from the previously finished chunk.
                # Each chain's epilogue (denominator matmul + reciprocal +
                # normalize) is lagged into the next chain so the PE never
                # waits on the DVE accumulation at head boundaries.
                qc_order = list(range(1, n_sc)) + [0]
                epi_pend = []

                def flush_epi():
                    while epi_pend:
                        outp, lall, h, q0 = epi_pend.pop(0)
                        lrep = ps_t.tile(
                            [128, 2, SC], f32, tag="st", name="lrep"
                        )
                        nc.tensor.matmul(
                            lrep[:, 0, :], lhsT=ones, rhs=lall,
                            start=True, stop=True,
                        )
                        rec = p2r.tile([128, SC], f32, tag="rec")
                        nc.vector.reciprocal_approx_fast(rec, lrep[:, 0, :])
                        nc.vector.tensor_mul(uT[:, h, q0 : q0 + SC], outp, rec)

                for oi, qc in enumerate(qc_order):
                    q0 = qc * SC
                    nfull = 4 * qc          # full (sub-diagonal) k-chunks
                    nkc = nfull + 4
                    prev_qc = qc_order[oi - 1] if oi > 0 else None
                    for h in range(HPC):
                        outp = ps_o.tile([128, SC], f32, tag="o")
                        lall = p2l.tile([128, SC], f16, tag="lp")
                        lst = [False]       # lall initialized?
                        pend_av = []        # (kc, pt AP, co) awaiting AV
                        og_gen = (
                            phase3_og_gen(prev_qc, h)
                            if prev_qc is not None
                            else None
                        )

                        def dose():
                            # a slice of the previous q-chunk's output
                            # projection as PE filler between score pairs
                            if og_gen is not None:
                                next(og_gen, None)

                        def mid_chain():
                            # previous chain's epilogue while this chain's
                            # first exps complete
                            flush_epi()
                            dose()

                        def lacc(ap, co):
                            # fp16 DVE accumulation of the softmax denominator
                            if not lst[0]:
                                nc.vector.tensor_copy(lall, ap)
                                lst[0] = True
                            else:
                                nc.vector.tensor_add(
                                    lall[:, co:], lall[:, co:], ap
                                )

                        def flush_av(upto):
                            # AV matmuls lag the score/exp stream to keep exp
                            # latency off the PE critical path
                            while len(pend_av) > upto:
                                kc, pt_ap, co = pend_av.pop(0)
                                nc.tensor.matmul(
                                    outp[:, co:],
                                    lhsT=vS[:, kc, h * HD : (h + 1) * HD],
                                    rhs=pt_ap,
                                    start=(kc == 0),
                                    stop=(kc == nkc - 1),
                                )

                        # --- full chunks, exp'd in pairs ---
                        for pr in range(nfull // 2):
                            st2 = ps_t.tile([128, 2, SC], f32, tag="st")
                            pt2 = p2.tile([128, 2, SC], f16, tag="pt")
                            for j in range(2):
                                kc = pr * 2 + j
                                nc.tensor.matmul(
                                    st2[:, j, :],
                                    lhsT=kT[:, h, kc * 128 : (kc + 1) * 128],
                                    rhs=qT[:, h, q0 : q0 + SC],
                                    start=True,
                                    stop=True,
                                )
                            nc.scalar.activation(pt2, st2, EXP, scale=scale)
                            if pr == 0:
                                mid_chain()
                            else:
                                dose()
                            if lst[0]:
                                nc.vector.tensor_add(lall, lall, pt2[:, 0, :])
                            else:
                                nc.vector.tensor_add(
                                    lall, pt2[:, 0, :], pt2[:, 1, :]
                                )
                                lst[0] = True
                            if pr > 0:
                                nc.vector.tensor_add(lall, lall, pt2[:, 1, :])
                            pend_av.append((pr * 2, pt2[:, 0, :], 0))
                            pend_av.append((pr * 2 + 1, pt2[:, 1, :], 0))
                            flush_av(3)

                        # --- diagonal chunks, trimmed + masked ---
                        for di in range(4):
                            kc = nfull + di
                            co = 128 * di
                            st2 = ps_t.tile([128, 2, SC], f32, tag="st")
                            pt2 = p2.tile([128, 2, SC], f16, tag="pt")
                            nc.tensor.matmul(
                                st2[:, 0, co:],
                                lhsT=kT[:, h, kc * 128 : (kc + 1) * 128],
                                rhs=qT[:, h, q0 + co : q0 + SC],
                                start=True,
                                stop=True,
                            )
                            nc.scalar.activation(
                                pt2[:, 0, co:], st2[:, 0, co:], EXP, scale=scale
                            )
                            if di == 0 and nfull == 0:
                                mid_chain()
                            else:
                                dose()
                            nc.vector.tensor_mul(
                                pt2[:, 0, co : co + 128],
                                pt2[:, 0, co : co + 128],
                                tri01,
                            )
                            lacc(pt2[:, 0, co:], co)
                            pend_av.append((kc, pt2[:, 0, co:], co))
                            flush_av(3)
                        flush_av(0)
                        if og_gen is not None:
                            for _ in og_gen:
                                pass
                        epi_pend.append((outp, lall, h, q0))
                flush_epi()
                for og in range(n_oc // 4):
                    phase3_og(qc_order[-1], og)

    nc.compile()
    return nc


def make_in_maps(x, Wq, Wk, Wv, Wo):
    cosT, sinT = _rope_tables_T(S, HD)
    rT = _rot_matrix_T(HD)
    ones = np.ones((HD, HD), dtype=np.float16)
    tri = _tri01()
    n_din, n_sc = DIM // 128, S // SC
    xts = []
    for g in range(DP):
        xT = x[g].T.astype(np.float16)                      # [din, s]
        xts.append(np.ascontiguousarray(
            xT.reshape(n_din, 128, n_sc, SC).transpose(1, 2, 0, 3)
        ))                                                  # [128, si, c, j]
    in_maps = []
    for c in range(N_CORES):
        g, r = divmod(c, TP)
        sl = slice(r * DLOC, (r + 1) * DLOC)

        def tile_w_h(W):
            # [p, h, c, d] = W.T[c*128+p, h*128+d]
            wT = W[sl, :].T.astype(np.float16)              # [din, dloc]
            return np.ascontiguousarray(
                wT.reshape(n_din, 128, HPC, HD).transpose(1, 2, 0, 3)
            )

        def tile_w_c(W):
            wT = W[sl, :].T.astype(np.float16)              # [din, dloc]
            return np.ascontiguousarray(
                wT.reshape(n_din, 128, DLOC).transpose(1, 0, 2)
            )

        woT = Wo[:, sl].T.astype(np.float16)                # [dloc, dim]
        wo_t = np.ascontiguousarray(
            woT.reshape(HPC, 128, DIM).transpose(1, 0, 2)
        )
        in_maps.append(
            {
                "x": xts[g],
                "wq": tile_w_h(Wq),
                "wk": tile_w_h(Wk),
                "wv": tile_w_c(Wv),
                "wo": wo_t,
                "cosT": cosT,
                "sinT": sinT,
                "rT": rT,
                "ones": ones,
                "tri": tri,
            }
        )
    return in_maps


def kernel(x, Wq, Wk, Wv, Wo, _trace=False):
    """Full-input / full-output entry point. Shards over 8 cores internally."""
    if "/opt/trn_rl_repo" not in sys.path:
        sys.path.insert(0, "/opt/trn_rl_repo")
    from concourse.bass_utils import run_bass_kernel_spmd

    x = np.asarray(x, dtype=np.float32)
    Wq, Wk, Wv, Wo = (np.asarray(w, dtype=np.float32) for w in (Wq, Wk, Wv, Wo))

    key = (B, S, DIM)
    if key not in _PROGRAM_CACHE:
        _PROGRAM_CACHE[key] = build_program(S, DIM)
    nc = _PROGRAM_CACHE[key]

    in_maps = make_in_maps(x, Wq, Wk, Wv, Wo)
    res = run_bass_kernel_spmd(
        nc, in_maps, core_ids=list(range(N_CORES)), trace=_trace
    )
    kernel.last_results = res
    out = np.empty((B, S, DIM), dtype=np.float32)
    for g in range(DP):
        acc = res.results[g * TP]["out"].astype(np.float32)
        for r in range(1, TP):
            acc = acc + res.results[g * TP + r]["out"].astype(np.float32)
        # [128, qc, oc, j] -> [oc*128, qc*512]
        outT = acc.transpose(2, 0, 1, 3).reshape(DIM, S)
        out[g] = outT.T
    return out


# revision 35
# speedup vs baseline: 1.0056x; 1.0056x over previous
"""Multi-head causal self-attention with RoPE on 8 Trainium2 NeuronCores.

Sharding: DP(2) x TP(4). Cores [4g, 4g+4) own batch g; within a group,
core r owns heads [4r, 4r+4) (rows [r*512,(r+1)*512) of Wq/Wk/Wv and the
matching columns of Wo). The host sums the 4 partial output projections
per batch (replaces the TP all-reduce); partial sums travel as fp16.

Performance notes (measured on TRN2):
  - PE matmul issue rate is N cycles @2.4GHz regardless of operand dtype
    (fp16 == bf16 == weight-reuse); the kernel is PE-streaming-bound, so
    everything else is organized to keep the PE FIFO dense.
  - dma_start issue on the Sync engine costs ~2.7ns per descriptor line;
    all DRAM tensors are pre-tiled on the host so every transfer is 128
    long per-partition-contiguous descriptors (~0.35us issue each).
  - Scalar activations pay a ~370-cycle access-latency adder, so exps are
    batched two k-chunks per call (st pairs span 2 PSUM banks; engines
    other than the PE may read across banks).
  - Softmax denominators: all exp'd chunks of a (head, q-chunk) chain
    are accumulated in fp16 on the DVE (2x mode), then partition-reduced
    with a single ones-matmul; the reduce/reciprocal/normalize epilogue
    is lagged into the next chain to stay off the PE critical path.
  - Causality: diagonal-band score chunks are column-trimmed to
    N = 512-128*di and masked multiplicatively (after exp) with a single
    [128,128] triangular 0/1 mask.
"""

import sys

import numpy as np

B, S, DIM = 2, 2048, 2048
NUM_HEADS = 16
HD = 128
N_CORES = 8
DP = 2                       # data-parallel groups (one batch each)
TP = N_CORES // DP           # tensor-parallel ranks per group
HPC = NUM_HEADS // TP        # heads per core (4)
DLOC = HPC * HD              # per-core slice of the model dim (512)
ROPE_BASE = 10000.0
SC = 512                     # s-chunk for projections / attention q-chunk

_PROGRAM_CACHE = {}


def _rope_tables_T(seq_len, head_dim):
    # match reference float32 arithmetic: inv_freq over even indices,
    # emb = cat(freqs, freqs); returned transposed [head_dim, seq_len]
    inv_freq = (
        1.0
        / (np.float32(ROPE_BASE)
           ** (np.arange(0, head_dim, 2, dtype=np.float32) / np.float32(head_dim)))
    ).astype(np.float32)
    t = np.arange(seq_len, dtype=np.float32)
    freqs = np.outer(t, inv_freq).astype(np.float32)      # [S, D/2]
    emb = np.concatenate([freqs, freqs], axis=-1)         # [S, D]
    return (
        np.ascontiguousarray(np.cos(emb).astype(np.float16).T),
        np.ascontiguousarray(np.sin(emb).astype(np.float16).T),
    )


def _rot_matrix_T(head_dim):
    # rotated = cat(-x[1::2], x[::2]) = R @ x; return R.T [D, D]
    d2 = head_dim // 2
    R = np.zeros((head_dim, head_dim), dtype=np.float16)
    for dp in range(d2):
        R[dp, 2 * dp + 1] = -1.0
    for dp in range(d2, head_dim):
        R[dp, 2 * (dp - d2)] = 1.0
    return np.ascontiguousarray(R.T)


def _tri01():
    # tri01[kk, qq] = 1 if kk <= qq else 0 (multiplicative causal mask for
    # the [128,128] diagonal block of every diagonal k-chunk)
    kk = np.arange(128)[:, None]
    qq = np.arange(128)[None, :]
    return np.ascontiguousarray((kk <= qq).astype(np.float16))


def build_program(s=S, dim=DIM):
    """Per-core SPMD Bass program (identical on every core)."""
    if "/opt/trn_rl_repo" not in sys.path:
        sys.path.insert(0, "/opt/trn_rl_repo")
    import concourse.bacc as bacc
    import concourse.mybir as mybir
    import concourse.tile as tile

    f32 = mybir.dt.float32
    f16 = mybir.dt.float16
    EXP = mybir.ActivationFunctionType.Exp

    n_din = dim // 128          # contraction chunks for projections (16)
    n_sc = s // SC              # s-chunks (4)
    n_oc = dim // 128           # output-projection row chunks (16)
    scale = float(HD) ** -0.5

    nc = bacc.Bacc("TRN2", target_bir_lowering=False, debug=False)

    # all DRAM tensors pre-tiled on the host: partition dim first, then
    # per-partition-contiguous free dims, so DMAs are 128 fat descriptors
    x_d = nc.dram_tensor("x", [128, n_sc, n_din, SC], f16, kind="ExternalInput")
    wq_d = nc.dram_tensor("wq", [128, HPC, n_din, HD], f16, kind="ExternalInput")
    wk_d = nc.dram_tensor("wk", [128, HPC, n_din, HD], f16, kind="ExternalInput")
    wv_d = nc.dram_tensor("wv", [128, n_din, DLOC], f16, kind="ExternalInput")
    wo_d = nc.dram_tensor("wo", [128, HPC, dim], f16, kind="ExternalInput")
    cosT_d = nc.dram_tensor("cosT", [HD, s], f16, kind="ExternalInput")
    sinT_d = nc.dram_tensor("sinT", [HD, s], f16, kind="ExternalInput")
    rT_d = nc.dram_tensor("rT", [HD, HD], f16, kind="ExternalInput")
    ones_d = nc.dram_tensor("ones", [HD, HD], f16, kind="ExternalInput")
    tri_d = nc.dram_tensor("tri", [HD, HD], f16, kind="ExternalInput")
    out_d = nc.dram_tensor("out", [128, n_sc, n_oc, SC], f16, kind="ExternalOutput")

    with tile.TileContext(nc) as tc:
        with tc.tile_pool(name="persist", bufs=1) as persist:
            qT = persist.tile([128, HPC, s], f16)   # roped q, [d, h, s]
            kT = persist.tile([128, HPC, s], f16)
            vS = persist.tile([128, s // 128, DLOC], f16)  # [k, chunk, d]
            uT = persist.tile([128, HPC, s], f16)   # attention out, [d, h, s]

            # ---------------- phase 1: qkv projections + RoPE ----------------
            with (
                tc.tile_pool(name="p1x", bufs=3) as p1x,
                tc.tile_pool(name="p1w", bufs=1) as p1w,
                tc.tile_pool(name="p1t", bufs=2) as p1t,
                tc.tile_pool(name="ps1", bufs=2, space="PSUM") as ps1,
            ):
                # startup criticality order: chain (q,h=0) is paced by xt0's
                # c-groups and wq's h=0 slice; later pieces arrive while
                # earlier chains compute
                xts = [
                    p1x.tile([128, n_din, SC], f16, tag="xt", name=f"xt{si}")
                    for si in range(3)
                ]
                wq_s = p1w.tile([128, HPC, n_din, HD], f16)
                wk_s = p1w.tile([128, HPC, n_din, HD], f16)
                wv_s = p1w.tile([128, n_din, DLOC], f16)
                rTs = persist.tile([HD, HD], f16)
                cosT = persist.tile([HD, s], f16)
                sinT = persist.tile([HD, s], f16)
                gq = n_din // 4
                nc.sync.dma_start(out=xts[0][:, :gq, :], in_=x_d[:, 0, :gq, :])
                nc.sync.dma_start(out=wq_s[:, 0, :, :], in_=wq_d[:, 0, :, :])
                for g0 in range(gq, n_din, gq):
                    sl4 = slice(g0, g0 + gq)
                    nc.sync.dma_start(out=xts[0][:, sl4, :], in_=x_d[:, 0, sl4, :])
                for h in range(1, HPC):
                    nc.sync.dma_start(out=wq_s[:, h, :, :], in_=wq_d[:, h, :, :])
                nc.sync.dma_start(out=rTs, in_=rT_d[:])
                nc.sync.dma_start(out=cosT, in_=cosT_d[:])
                nc.sync.dma_start(out=sinT, in_=sinT_d[:])
                for h in range(HPC):
                    nc.sync.dma_start(out=wk_s[:, h, :, :], in_=wk_d[:, h, :, :])
                nh = n_din // 2
                nc.sync.dma_start(out=xts[1][:, :nh, :], in_=x_d[:, 1, :nh, :])
                nc.sync.dma_start(out=xts[1][:, nh:, :], in_=x_d[:, 1, nh:, :])
                nc.sync.dma_start(out=wv_s[:, :nh, :], in_=wv_d[:, :nh, :])
                nc.sync.dma_start(out=wv_s[:, nh:, :], in_=wv_d[:, nh:, :])
                nc.sync.dma_start(out=xts[2][:, :nh, :], in_=x_d[:, 2, :nh, :])
                nc.sync.dma_start(out=xts[2][:, nh:, :], in_=x_d[:, 2, nh:, :])
                ones = persist.tile([HD, HD], f16)
                nc.sync.dma_start(out=ones, in_=ones_d[:])
                tri01 = persist.tile([HD, HD], f16)
                nc.sync.dma_start(out=tri01, in_=tri_d[:])
                woT_s = persist.tile([128, HPC, dim], f16)
                nc.sync.dma_start(out=woT_s, in_=wo_d[:])

                def finish_rope(raw, store, h, s0):
                    # rotate-half as a partition-permuting SBUF->SBUF DMA
                    # (engines cannot cross partitions; the DMA can), with
                    # the rotation signs folded into the host sin table --
                    # saves a 216ns PE matmul per chain and runs the sin
                    # multiply at 2x fp16 DVE rate. Emitted one chain late
                    # so nothing waits on the scalar-engine raw copy.
                    rawp = p1t.tile([128, SC], f16, tag="rp", bufs=2)
                    nc.sync.dma_start(out=rawp[0:64, :], in_=raw[1:128:2, :])
                    nc.sync.dma_start(out=rawp[64:128, :], in_=raw[0:128:2, :])
                    t1 = p1t.tile([128, SC], f16, tag="t1")
                    nc.vector.tensor_mul(t1, raw, cosT[:, s0 : s0 + SC])
                    t2 = p1t.tile([128, SC], f16, tag="t2")
                    nc.vector.tensor_mul(t2, rawp, sinT[:, s0 : s0 + SC])
                    nc.gpsimd.tensor_add(store[:, h, s0 : s0 + SC], t1, t2)

                pending = None
                for si in range(n_sc):
                    s0 = si * SC
                    if si < 3:
                        xt = xts[si]
                    else:
                        xt = p1x.tile([128, n_din, SC], f16, tag="xt", name="xt3")
                        nc.sync.dma_start(out=xt, in_=x_d[:, si, :, :])

                    for w_s, store in ((wq_s, qT), (wk_s, kT)):
                        for h in range(HPC):
                            acc = ps1.tile([128, SC], f32, tag="acc")
                            for c in range(n_din):
                                nc.tensor.matmul(
                                    acc,
                                    lhsT=w_s[:, h, c, :],
                                    rhs=xt[:, c, :],
                                    start=(c == 0),
                                    stop=(c == n_din - 1),
                                )
                            raw = p1t.tile([128, SC], f16, tag="raw", bufs=3)
                            nc.scalar.copy(raw, acc)
                            if pending is not None:
                                finish_rope(*pending)
                            pending = (raw, store, h, s0)

                    for sub in range(SC // 128):   # v chains
                        vacc = ps1.tile([128, SC], f32, tag="vacc")
                        for c in range(n_din):
                            nc.tensor.matmul(
                                vacc,
                                lhsT=xt[:, c, sub * 128 : (sub + 1) * 128],
                                rhs=wv_s[:, c, :],
                                start=(c == 0),
                                stop=(c == n_din - 1),
                            )
                        if pending is not None:
                            finish_rope(*pending)
                            pending = None
                        vdst = vS[:, si * 4 + sub, :]
                        if sub % 2 == 0:
                            nc.scalar.copy(vdst, vacc)
                        else:
                            nc.vector.tensor_copy(vdst, vacc)

            # ------------- phase 2+3: attention + output projection -------------
            with (
                tc.tile_pool(name="p2", bufs=8) as p2,
                tc.tile_pool(name="p2l", bufs=2) as p2l,
                tc.tile_pool(name="p2r", bufs=2) as p2r,
                tc.tile_pool(name="p3", bufs=2) as p3,
                tc.tile_pool(name="ps_t", bufs=3, space="PSUM") as ps_t,
                tc.tile_pool(name="ps_o", bufs=2, space="PSUM") as ps_o,
            ):
                def phase3_og_gen(qc, og):
                    # one 4-row-chunk group of the output projection for
                    # q-chunk qc: 16 PE matmuls + 2 psum-pair copies + 1 DMA.
                    # Yields after each 4-matmul row-chunk so the group can be
                    # dosed through the next q-chunk's attention chains --
                    # keeping per-pair PE work above the scalar exp rate.
                    q0 = qc * SC
                    ot4 = p3.tile([128, 4, SC], f16, tag="ot")
                    for op_ in range(2):
                        pos2 = ps_t.tile([128, 2, SC], f32, tag="st", name="pos")
                        for j in range(2):
                            oc = og * 4 + op_ * 2 + j
                            for h in range(HPC):
                                nc.tensor.matmul(
                                    pos2[:, j, :],
                                    lhsT=woT_s[:, h, oc * 128 : (oc + 1) * 128],
                                    rhs=uT[:, h, q0 : q0 + SC],
                                    start=(h == 0),
                                    stop=(h == HPC - 1),
                                )
                            # per-row-chunk psum evictions, alternating
                            # engines: half-size ops block the latency-
                            # critical exps/masks in the FIFOs half as long
                            dst = ot4[:, op_ * 2 + j, :]
                            if (op_ * 2 + j) % 2 == 0:
                                nc.scalar.copy(dst, pos2[:, j, :])
                            else:
                                nc.vector.tensor_copy(dst, pos2[:, j, :])
                            yield
                    nc.sync.dma_start(
                        out=out_d[:, qc, og * 4 : (og + 1) * 4, :], in_=ot4
                    )

                def phase3_og(qc, og):
                    for _ in phase3_og_gen(qc, og):
                        pass

                # attention q-chunks processed in rotated order so every
                # chain (including the short qc=0 ones) carries an output-
                # projection filler group from the previously finished chunk.
                # Each chain's epilogue (denominator matmul + reciprocal +
                # normalize) is lagged into the next chain so the PE never
                # waits on the DVE accumulation at head boundaries.
                qc_order = list(range(1, n_sc)) + [0]
                epi_pend = []

                def flush_epi():
                    while epi_pend:
                        outp, lall, h, q0 = epi_pend.pop(0)
                        lrep = ps_t.tile(
                            [128, 2, SC], f32, tag="st", name="lrep"
                        )
                        nc.tensor.matmul(
                            lrep[:, 0, :], lhsT=ones, rhs=lall,
                            start=True, stop=True,
                        )
                        rec = p2r.tile([128, SC], f32, tag="rec")
                        nc.vector.reciprocal_approx_fast(rec, lrep[:, 0, :])
                        nc.vector.tensor_mul(uT[:, h, q0 : q0 + SC], outp, rec)

                for oi, qc in enumerate(qc_order):
                    q0 = qc * SC
                    nfull = 4 * qc          # full (sub-diagonal) k-chunks
                    nkc = nfull + 4
                    prev_qc = qc_order[oi - 1] if oi > 0 else None
                    for h in range(HPC):
                        outp = ps_o.tile([128, SC], f32, tag="o")
                        lall = p2l.tile([128, SC], f16, tag="lp")
                        lst = [False]       # lall initialized?
                        pend_av = []        # (kc, pt AP, co) awaiting AV
                        og_gen = (
                            phase3_og_gen(prev_qc, h)
                            if prev_qc is not None
                            else None
                        )

                        def dose():
                            # a slice of the previous q-chunk's output
                            # projection as PE filler between score pairs
                            if og_gen is not None:
                                next(og_gen, None)

                        def mid_chain():
                            # previous chain's epilogue while this chain's
                            # first exps complete
                            flush_epi()
                            dose()

                        def lacc(ap, co):
                            # fp16 DVE accumulation of the softmax denominator
                            if not lst[0]:
                                nc.vector.tensor_copy(lall, ap)
                                lst[0] = True
                            else:
                                nc.vector.tensor_add(
                                    lall[:, co:], lall[:, co:], ap
                                )

                        def flush_av(upto):
                            # AV matmuls lag the score/exp stream to keep exp
                            # latency off the PE critical path
                            while len(pend_av) > upto:
                                kc, pt_ap, co = pend_av.pop(0)
                                nc.tensor.matmul(
                                    outp[:, co:],
                                    lhsT=vS[:, kc, h * HD : (h + 1) * HD],
                                    rhs=pt_ap,
                                    start=(kc == 0),
                                    stop=(kc == nkc - 1),
                                )

                        # --- full chunks, exp'd in pairs ---
                        for pr in range(nfull // 2):
                            st2 = ps_t.tile([128, 2, SC], f32, tag="st")
                            pt2 = p2.tile([128, 2, SC], f16, tag="pt")
                            for j in range(2):
                                kc = pr * 2 + j
                                nc.tensor.matmul(
                                    st2[:, j, :],
                                    lhsT=kT[:, h, kc * 128 : (kc + 1) * 128],
                                    rhs=qT[:, h, q0 : q0 + SC],
                                    start=True,
                                    stop=True,
                                )
                            nc.scalar.activation(pt2, st2, EXP, scale=scale)
                            if pr == 0:
                                mid_chain()
                            else:
                                dose()
                            if lst[0]:
                                nc.vector.tensor_add(lall, lall, pt2[:, 0, :])
                            else:
                                nc.vector.tensor_add(
                                    lall, pt2[:, 0, :], pt2[:, 1, :]
                                )
                                lst[0] = True
                            if pr > 0:
                                nc.vector.tensor_add(lall, lall, pt2[:, 1, :])
                            pend_av.append((pr * 2, pt2[:, 0, :], 0))
                            pend_av.append((pr * 2 + 1, pt2[:, 1, :], 0))
                            flush_av(3)

                        # --- diagonal chunks, trimmed + masked ---
                        for di in range(4):
                            kc = nfull + di
                            co = 128 * di
                            st2 = ps_t.tile([128, 2, SC], f32, tag="st")
                            pt2 = p2.tile([128, 2, SC], f16, tag="pt")
                            nc.tensor.matmul(
                                st2[:, 0, co:],
                                lhsT=kT[:, h, kc * 128 : (kc + 1) * 128],
                                rhs=qT[:, h, q0 + co : q0 + SC],
                                start=True,
                                stop=True,
                            )
                            nc.scalar.activation(
                                pt2[:, 0, co:], st2[:, 0, co:], EXP, scale=scale
                            )
                            if di == 0 and nfull == 0:
                                mid_chain()
                            else:
                                dose()
                            nc.vector.tensor_mul(
                                pt2[:, 0, co : co + 128],
                                pt2[:, 0, co : co + 128],
                                tri01,
                            )
                            lacc(pt2[:, 0, co:], co)
                            pend_av.append((kc, pt2[:, 0, co:], co))
                            flush_av(3)
                        flush_av(0)
                        if og_gen is not None:
                            for _ in og_gen:
                                pass
                        epi_pend.append((outp, lall, h, q0))
                flush_epi()
                for og in range(n_oc // 4):
                    phase3_og(qc_order[-1], og)

    nc.compile()
    return nc


def make_in_maps(x, Wq, Wk, Wv, Wo):
    cosT, sinT = _rope_tables_T(S, HD)
    # rotate-half signs folded into the sin table: t2[d] = raw[perm(d)]*sin'[d]
    sinT_mod = sinT.copy()
    sinT_mod[:64] *= np.float16(-1)
    rT = _rot_matrix_T(HD)
    ones = np.ones((HD, HD), dtype=np.float16)
    tri = _tri01()
    n_din, n_sc = DIM // 128, S // SC
    xts = []
    for g in range(DP):
        xT = x[g].T.astype(np.float16)                      # [din, s]
        xts.append(np.ascontiguousarray(
            xT.reshape(n_din, 128, n_sc, SC).transpose(1, 2, 0, 3)
        ))                                                  # [128, si, c, j]
    in_maps = []
    for c in range(N_CORES):
        g, r = divmod(c, TP)
        sl = slice(r * DLOC, (r + 1) * DLOC)

        def tile_w_h(W):
            # [p, h, c, d] = W.T[c*128+p, h*128+d]
            wT = W[sl, :].T.astype(np.float16)              # [din, dloc]
            return np.ascontiguousarray(
                wT.reshape(n_din, 128, HPC, HD).transpose(1, 2, 0, 3)
            )

        def tile_w_c(W):
            wT = W[sl, :].T.astype(np.float16)              # [din, dloc]
            return np.ascontiguousarray(
                wT.reshape(n_din, 128, DLOC).transpose(1, 0, 2)
            )

        woT = Wo[:, sl].T.astype(np.float16)                # [dloc, dim]
        wo_t = np.ascontiguousarray(
            woT.reshape(HPC, 128, DIM).transpose(1, 0, 2)
        )
        in_maps.append(
            {
                "x": xts[g],
                "wq": tile_w_h(Wq),
                "wk": tile_w_h(Wk),
                "wv": tile_w_c(Wv),
                "wo": wo_t,
                "cosT": cosT,
                "sinT": sinT_mod,
                "rT": rT,
                "ones": ones,
                "tri": tri,
            }
        )
    return in_maps


def kernel(x, Wq, Wk, Wv, Wo, _trace=False):
    """Full-input / full-output entry point. Shards over 8 cores internally."""
    if "/opt/trn_rl_repo" not in sys.path:
        sys.path.insert(0, "/opt/trn_rl_repo")
    from concourse.bass_utils import run_bass_kernel_spmd

    x = np.asarray(x, dtype=np.float32)
    Wq, Wk, Wv, Wo = (np.asarray(w, dtype=np.float32) for w in (Wq, Wk, Wv, Wo))

    key = (B, S, DIM)
    if key not in _PROGRAM_CACHE:
        _PROGRAM_CACHE[key] = build_program(S, DIM)
    nc = _PROGRAM_CACHE[key]

    in_maps = make_in_maps(x, Wq, Wk, Wv, Wo)
    res = run_bass_kernel_spmd(
        nc, in_maps, core_ids=list(range(N_CORES)), trace=_trace
    )
    kernel.last_results = res
    out = np.empty((B, S, DIM), dtype=np.float32)
    for g in range(DP):
        acc = res.results[g * TP]["out"].astype(np.float32)
        for r in range(1, TP):
            acc = acc + res.results[g * TP + r]["out"].astype(np.float32)
        # [128, qc, oc, j] -> [oc*128, qc*512]
        outT = acc.transpose(2, 0, 1, 3).reshape(DIM, S)
        out[g] = outT.T
    return out
